# revision 7
# baseline (speedup 1.0000x reference)
"""Trainium2 Bass kernel for single-head causal attention.

Problem: x[4, 2048, 1024] fp32; wq/wk/wv [1024, 1024] (torch layout [d_out, d_in]).
  q = x @ wq.T ; k = x @ wk.T ; v = x @ wv.T  (per batch)
  out = softmax(causal(q @ k.T) / 32) @ v

Sharding (8 cores): core c = 2*b + h owns batch b and half of its query rows.
Query rows are split between the two cores of a batch by parity *within* each
512-row group so both cores see an identical causal work profile -> the SPMD
program is fully uniform; only data (inputs) differ per core.

Per-core device program (fp32r matmuls, all N >= 256 so PE runs at full rate):
  phase 1a: kT[o, s] = wkT-chunks.T @ xkvT  (k transposed layout, SBUF-resident)
  phase 1b: v[s, o]  = xkvT-chunks.T @ wvT  -> spilled to DRAM scratch
  phase 1c: qT[o, s_own] = wqT-chunks.T @ xqT -> spilled to DRAM scratch
  phase 2:  per group g (4 groups of 256 own-q columns, kb = 4g+4 key blocks):
      scoresT[k, q'] = sum_o kT.T @ qT   (PSUM, 8 o-chunk matmuls)
      p = exp(scores/32)  (ACT, PSUM->SBUF); causal mask multiply (DVE) on
      the last 4 key blocks; l[q'] += ones.T @ p (PE); AV accumulation
      out[q',o] += p.T-slices @ v-blocks; final ACT copy divides by l via a
      per-partition scale AP (l transposed into [128,1] lanes by a tiny DMA).
"""

import os
import sys
import types
from contextlib import ExitStack

for _p in ("/opt/trn_rl_repo", "/root/.axon_site/_ro/trn_rl_repo"):
    if os.path.isdir(_p) and _p not in sys.path:
        sys.path.insert(0, _p)

import numpy as np

import concourse.bacc as bacc
import concourse.mybir as mybir
import concourse.tile as tile
from concourse.bass_utils import run_bass_kernel_spmd

F32 = mybir.dt.float32
F32R = mybir.dt.float32r

B, S, D = 4, 2048, 1024
P = 128
DC = D // P      # 8 contraction chunks
OC = D // P      # 8 output-dim chunks
SKB = S // P     # 16 key blocks
G = 4            # query groups per core
QW = 256         # query columns per group per core
SQ = G * QW      # 1024 own query rows per core
N_CORES = 8
SCALE = 1.0 / 32.0  # 1/sqrt(D)


def _install_axon_profile_hook():
    """Provide antenv.axon_hooks (absent in this image) so trace=True works."""
    name = "antenv.axon_hooks"
    if name in sys.modules:
        return
    mod = types.ModuleType(name)
    _hook = [None]
    mod.set_axon_ntff_profile_hook = lambda h: _hook.__setitem__(0, h)
    mod.get_axon_ntff_profile_hook = lambda: _hook[0]
    sys.modules[name] = mod
    try:
        import antenv

        antenv.axon_hooks = mod
        from trn_agent_boot.trn_boot import _ntff_profile_via_ctypes

        mod.set_axon_ntff_profile_hook(
            _ntff_profile_via_ctypes("/opt/axon/libaxon_pjrt.so")
        )
    except Exception:
        pass


def _round_fp32r(a):
    """Round fp32 to fp32r (11 stored mantissa bits, RNE) as the PE expects."""
    u = np.ascontiguousarray(a, dtype=np.float32).view(np.uint32)
    r = (u + np.uint32(0x7FF) + ((u >> np.uint32(12)) & np.uint32(1))) \
        & np.uint32(0xFFFFF000)
    return r.view(np.float32)


def _build_program():
    nc = bacc.Bacc("TRN2", target_bir_lowering=False, debug=False,
                   num_devices=N_CORES)

    xkv = nc.dram_tensor("xkv", [D, S], F32R, kind="ExternalInput").ap()
    xq = nc.dram_tensor("xq", [D, SQ], F32R, kind="ExternalInput").ap()
    wqt = nc.dram_tensor("wqt", [D, D], F32R, kind="ExternalInput").ap()
    wkt = nc.dram_tensor("wkt", [D, D], F32R, kind="ExternalInput").ap()
    wvt = nc.dram_tensor("wvt", [D, D], F32R, kind="ExternalInput").ap()
    ones_in = nc.dram_tensor("ones", [P, 1], F32R, kind="ExternalInput").ap()
    mask = nc.dram_tensor("mask", [P, 16 * QW], F32, kind="ExternalInput").ap()
    out = nc.dram_tensor("out", [SQ, D], F32, kind="ExternalOutput").ap()
    vdram = nc.dram_tensor("vscratch", [S, D], F32R).ap()
    qtdram = nc.dram_tensor("qtscratch", [D, SQ], F32R).ap()

    with tile.TileContext(nc, pool_alloc_mode="queue") as tc, ExitStack() as es:
        const = es.enter_context(tc.tile_pool(name="const", bufs=1))
        ktpool = es.enter_context(tc.tile_pool(name="ktpool", bufs=8))

        ones1 = const.tile([P, 1], F32R)
        nc.sync.dma_start(out=ones1[:], in_=ones_in[:])
        mask_sb = const.tile([P, 16 * QW], F32)
        nc.sync.dma_start(out=mask_sb[:], in_=mask[:])

        p1 = ExitStack()
        # weight half-chunks [128, 512]; one shared pool cycles wk -> wv -> wq
        wpool = p1.enter_context(tc.tile_pool(name="wpool", bufs=24))
        pp = p1.enter_context(tc.tile_pool(name="pp", bufs=2, space="PSUM"))

        def load_w(src, dc, hf):
            w = wpool.tile([P, 512], F32R, tag="w")
            nc.sync.dma_start(
                out=w[:], in_=src[dc * P:(dc + 1) * P, hf * 512:(hf + 1) * 512])
            return w

        wk_sb = [[load_w(wkt, dc, hf) for dc in range(DC)] for hf in range(2)]

        x_es = ExitStack()
        xkvpool = x_es.enter_context(tc.tile_pool(name="xkvpool", bufs=8))
        vstage = x_es.enter_context(tc.tile_pool(name="vstage", bufs=3))
        xkv_sb = []
        for dc in range(DC):
            t = xkvpool.tile([P, S], F32R, tag="xkv")
            nc.sync.dma_start(out=t[:], in_=xkv[dc * P:(dc + 1) * P, :])
            xkv_sb.append(t)

        # ---- phase 1a: kT projection (resident) ----
        kt = []
        for oc in range(OC):
            hf, ocr = divmod(oc, 4)
            ktile = ktpool.tile([P, S], F32R, tag="kt")
            for st in range(S // 512):
                ps = pp.tile([P, 512], F32, tag="pp")
                for dc in range(DC):
                    nc.tensor.matmul(
                        ps[:],
                        wk_sb[hf][dc][:, ocr * P:(ocr + 1) * P],
                        xkv_sb[dc][:, st * 512:(st + 1) * 512],
                        start=(dc == 0), stop=(dc == DC - 1),
                    )
                nc.vector.tensor_copy(ktile[:, st * 512:(st + 1) * 512], ps[:])
            kt.append(ktile)

        # ---- phase 1b: v projection -> DRAM spill ----
        wv_sb = [[load_w(wvt, dc, ot) for dc in range(DC)] for ot in range(2)]
        for ot in range(2):
            for sb in range(SKB):
                ps = pp.tile([P, 512], F32, tag="pp")
                for dc in range(DC):
                    nc.tensor.matmul(
                        ps[:],
                        xkv_sb[dc][:, sb * P:(sb + 1) * P],
                        wv_sb[ot][dc][:],
                        start=(dc == 0), stop=(dc == DC - 1),
                    )
                vt = vstage.tile([P, 512], F32R, tag="v")
                nc.vector.tensor_copy(vt[:], ps[:])
                nc.scalar.dma_start(
                    out=vdram[sb * P:(sb + 1) * P, ot * 512:(ot + 1) * 512],
                    in_=vt[:])
        x_es.close()

        # ---- phase 1c: qT projection -> DRAM spill ----
        wq_sb = [[load_w(wqt, dc, hf) for dc in range(DC)] for hf in range(2)]
        xq_es = ExitStack()
        xqpool = xq_es.enter_context(tc.tile_pool(name="xqpool", bufs=8))
        qstage = xq_es.enter_context(tc.tile_pool(name="qstage", bufs=3))
        xq_sb = []
        for dc in range(DC):
            t = xqpool.tile([P, SQ], F32R, tag="xq")
            nc.sync.dma_start(out=t[:], in_=xq[dc * P:(dc + 1) * P, :])
            xq_sb.append(t)
        for oc in range(OC):
            hf, ocr = divmod(oc, 4)
            for st in range(SQ // 512):
                ps = pp.tile([P, 512], F32, tag="pp")
                for dc in range(DC):
                    nc.tensor.matmul(
                        ps[:],
                        wq_sb[hf][dc][:, ocr * P:(ocr + 1) * P],
                        xq_sb[dc][:, st * 512:(st + 1) * 512],
                        start=(dc == 0), stop=(dc == DC - 1),
                    )
                qst = qstage.tile([P, 512], F32R, tag="q")
                nc.vector.tensor_copy(qst[:], ps[:])
                nc.scalar.dma_start(
                    out=qtdram[oc * P:(oc + 1) * P, st * 512:(st + 1) * 512],
                    in_=qst[:])
        xq_es.close()
        p1.close()

        # ---- phase 2: attention ----
        vpool = es.enter_context(tc.tile_pool(name="vpool", bufs=16))
        qspool = es.enter_context(tc.tile_pool(name="qspool", bufs=16))
        ptpool = es.enter_context(tc.tile_pool(name="ptpool", bufs=18))
        linvpool = es.enter_context(tc.tile_pool(name="linvpool", bufs=2))
        linvtpool = es.enter_context(tc.tile_pool(name="linvtpool", bufs=2))
        outpool = es.enter_context(tc.tile_pool(name="outpool", bufs=2))
        ps_s = es.enter_context(tc.tile_pool(name="ps_s", bufs=2, space="PSUM"))
        ps_l = es.enter_context(tc.tile_pool(name="ps_l", bufs=2, space="PSUM"))
        ps_av = es.enter_context(tc.tile_pool(name="ps_av", bufs=4, space="PSUM"))

        v_sb = []
        for kb in range(SKB):
            t = vpool.tile([P, D], F32R, tag="vres")
            nc.sync.dma_start(out=t[:], in_=vdram[kb * P:(kb + 1) * P, :])
            v_sb.append(t)

        for g in range(G):
            U = 4 * g + 4
            q_sb = []
            for oc in range(OC):
                t = qspool.tile([P, QW], F32R, tag="qs")
                nc.sync.dma_start(
                    out=t[:],
                    in_=qtdram[oc * P:(oc + 1) * P, g * QW:(g + 1) * QW])
                q_sb.append(t)

            l_ps = ps_l.tile([1, QW], F32, tag="l")
            pts = [None] * U
            score_ps = [None] * U

            def emit_post(j, g=g, U=U, l_ps=l_ps, pts=pts, score_ps=score_ps):
                # (additive causal mask) + exp + l-accumulation for unit j
                if j >= 4 * g:
                    nc.vector.tensor_add(
                        score_ps[j][:], score_ps[j][:],
                        mask_sb[:, j * QW:(j + 1) * QW])
                pt = ptpool.tile([P, QW], F32R, tag="pt")
                nc.scalar.activation(
                    pt[:], score_ps[j][:],
                    mybir.ActivationFunctionType.Exp, scale=SCALE)
                nc.tensor.matmul(
                    l_ps[:], ones1[:], pt[:],
                    start=(j == 0), stop=(j == U - 1),
                )
                pts[j] = pt

            for j in range(U):
                ps = ps_s.tile([P, QW], F32, tag="s")
                for oc in range(OC):
                    nc.tensor.matmul(
                        ps[:],
                        kt[oc][:, j * P:(j + 1) * P],
                        q_sb[oc][:],
                        start=(oc == 0), stop=(oc == OC - 1),
                    )
                score_ps[j] = ps
                if j >= 1:
                    emit_post(j - 1)
            emit_post(U - 1)

            # 1/l, transposed into per-partition lanes [128, 2]
            linv = linvpool.tile([1, QW], F32, tag="linv")
            nc.vector.reciprocal(linv[:], l_ps[:])
            linv_t = linvtpool.tile([P, 2], F32, tag="linvt")
            for c in range(2):
                nc.scalar.dma_start(
                    out=linv_t[:, c:c + 1], in_=linv[0:1, c * P:(c + 1) * P])

            av = {}
            for qs in range(2):
                for ot in range(2):
                    ps = ps_av.tile([P, 512], F32, tag="av")
                    for j in range(U):
                        nc.tensor.matmul(
                            ps[:],
                            pts[j][:, qs * P:(qs + 1) * P],
                            v_sb[j][:, ot * 512:(ot + 1) * 512],
                            start=(j == 0), stop=(j == U - 1),
                        )
                    av[qs, ot] = ps
            for qs in range(2):
                out_sb = outpool.tile([P, D], F32, tag="out")
                for ot in range(2):
                    nc.scalar.mul(
                        out_sb[:, ot * 512:(ot + 1) * 512],
                        av[qs, ot][:], linv_t[:, qs:qs + 1])
                r0 = g * 2 * P + qs * P
                nc.sync.dma_start(out=out[r0:r0 + P, :], in_=out_sb[:])

    nc.compile()
    return nc


_PROGRAM = None


def _get_program():
    global _PROGRAM
    if _PROGRAM is None:
        _PROGRAM = _build_program()
    return _PROGRAM


# Set by kernel() after each run: BassKernelResults (exec_time_ns etc.)
last_results = None


def kernel(**inputs):
    global last_results
    _install_axon_profile_hook()

    x = np.asarray(inputs["x"], dtype=np.float32)
    wq = np.asarray(inputs["wq"], dtype=np.float32)
    wk = np.asarray(inputs["wk"], dtype=np.float32)
    wv = np.asarray(inputs["wv"], dtype=np.float32)

    wqt = _round_fp32r(wq.T)
    wkt = _round_fp32r(wk.T)
    wvt = _round_fp32r(wv.T)

    # own query rows per core half h: parity-h rows within each 512-row group
    own_rows = {}
    for h in range(2):
        rows = []
        for g in range(G):
            rows.extend(range(512 * g + h, 512 * (g + 1), 2))
        own_rows[h] = np.array(rows, dtype=np.int64)

    # causal mask tiles [128, 16*256]: tile t covers key block t for group t//4
    masks = {}
    kl = np.arange(P)[:, None]
    qp = np.arange(QW)[None, :]
    for h in range(2):
        m = np.zeros((P, 16 * QW), dtype=np.float32)
        for t in range(16):
            g = t // 4
            krow = P * t + kl
            qrow = 512 * g + 2 * qp + h
            m[:, t * QW:(t + 1) * QW] = np.where(krow <= qrow, 0.0, -1e9)
        masks[h] = m

    in_maps = []
    for c in range(N_CORES):
        b, h = divmod(c, 2)
        xt = _round_fp32r(x[b].T)  # [D, S]
        in_maps.append({
            "xkv": xt,
            "xq": np.ascontiguousarray(xt[:, own_rows[h]]),
            "wqt": wqt, "wkt": wkt, "wvt": wvt,
            "mask": masks[h],
            "ones": np.ones((P, 1), dtype=np.float32),
        })

    nc = _get_program()
    trace = bool(int(os.environ.get("KERNEL_TRACE", "0")))
    kwargs = {}
    if trace:
        kwargs["trace"] = True
        kwargs["trace_cores"] = list(range(N_CORES))
    res = run_bass_kernel_spmd(nc, in_maps, core_ids=list(range(N_CORES)),
                               **kwargs)
    last_results = res

    out = np.empty((B, S, D), dtype=np.float32)
    for c in range(N_CORES):
        b, h = divmod(c, 2)
        out[b, own_rows[h], :] = res.results[c]["out"]
    return out
